# revision 17
# baseline (speedup 1.0000x reference)
"""Trainium2 Bass kernel for nn_Net_5428838662222 (dense_transformer).

Self-contained: takes FULL inputs (as produced by setup_inputs), returns FULL
[32768, 11] fp32 output. Data-parallel over 8 NeuronCores (4096 items each).

Algorithm (validated vs reference at ~1e-6 rel err in fp64/fp32 numpy):
  The embedding+input-proj output x[b,l,:] depends only on (tokens[b,l], l),
  i.e. on one of NIDX = 6*7 = 42 indices. Host precomputes tiny tables:
    T   [42,64]   input reps (emb ++ pos) @ in_w + in_b
    E_h [42,42]   exp(T Mh T^T / sqrt(D) - gmax),  Mh = wq_h wk_h^T
    VW_h[42,64]   (T @ wv_h) @ wo_h-slice
  Encoder MHA with softmax over the QUERY axis collapses to
    attn[b,q,:] = sum_v E_h[i(b,q),v] * cnt[b,v]/W_h[b,v] * VW_h[v,:]
  where cnt[b,v] is the item's index histogram and W_h = cnt @ E_h.  On-chip
  (feature-major: features on partitions, tokens on the free dim; within a
  tile, tokens are laid out l-major: column s = l*BT + b):
    oh   = one-hot(idx)            [42, Ntok]   (DVE compare vs iota)
    cnt  = seg-reduce oh over L    [42, Nitem]
    be   = stk^T-matmul [oh|cnt]   -> Erow and W in one PE pass per head-pair
    P    = Erow * (cnt/W) bcast    [84, Ntok] per head-pair
    y    = VW^T-matmul P (+accum)  [64, Ntok], + bo
  then LN1 (stats via ones-matmul broadcast to all partitions), FFN with
  folded LN1 affine, residual, LN2, per-item reduce over L, and the fused
  decoder+fc matmul (decoder softmax is over a singleton query axis ==
  all-ones, so the decoder collapses to xs @ Wdec; Wdec/fc/LN2-affine fold
  into one [64,11] matmul).
"""
import sys

for _p in ("/opt/trn_rl_repo",):
    if _p not in sys.path:
        sys.path.insert(0, _p)

import numpy as np

import concourse.bass as bass
import concourse.bacc as bacc
import concourse.tile as tile
from concourse import mybir
from concourse.alu_op_type import AluOpType as ALU
from concourse.bass_utils import run_bass_kernel_spmd
import bass_rust

AX = bass_rust.AxisListType
AF = mybir.ActivationFunctionType
dt = mybir.dt

# ---------------- problem constants ----------------
H, D, L, VOCAB, NOUT, EPS = 4, 64, 7, 6, 11, 1e-5
B, NCORES = 32768, 8
BC = B // NCORES          # items per core = 4096
BT = 64                   # items per tile
NT = BT * L               # tokens per tile = 448
NTILES = BC // BT         # 64
NIDX = VOCAB * L          # 42
D2 = 2 * D                # FFN hidden = 128

MMDT = dt.float32r        # matmul operand dtype (full-rate fp32 on PE at N>=256)
F32 = dt.float32


# ---------------- host-side table construction ----------------
def build_tables(inp: dict) -> dict:
    f64 = lambda k: np.asarray(inp[k], np.float64)
    emb, pos, in_w, in_b = f64('emb'), f64('pos'), f64('in_w'), f64('in_b')
    # T[l*VOCAB+v] = concat(emb[v], pos[l]) @ in_w + in_b
    T = np.zeros((NIDX, D))
    for l in range(L):
        for v in range(VOCAB):
            T[l * VOCAB + v] = np.concatenate([emb[v], pos[l]]) @ in_w + in_b
    e_wq, e_wk, e_wv, e_wo = f64('e_wq'), f64('e_wk'), f64('e_wv'), f64('e_wo')
    G = np.einsum('id,hde,hfe,jf->hij', T, e_wq, e_wk, T) / np.sqrt(float(D))
    E = np.exp(G - G.max())                                   # [H,42,42]
    V = np.einsum('id,hde->hie', T, e_wv)                     # [H,42,64]
    VW = np.stack([V[h] @ e_wo[h * D:(h + 1) * D] for h in range(H)])  # [H,42,64]
    g1, b1 = f64('e_g1'), f64('e_b1')
    c1w = g1[:, None] * f64('e_c1w')                          # fold LN1 scale
    c1b = b1 @ f64('e_c1w') + f64('e_c1b')                    # fold LN1 shift
    Wdec = np.concatenate([f64('d_wv')[h] for h in range(H)], axis=1) @ f64('d_wo')
    Wf_raw = Wdec @ f64('fc_w')
    Wf = f64('e_g2')[:, None] * Wf_raw                        # fold LN2 scale
    bf = f64('d_bo') @ f64('fc_w') + f64('fc_b') + L * (f64('e_b2') @ Wf_raw)

    ones_ln = np.zeros((2 * D, 2 * D))
    ones_ln[:D, :D] = 1.0 / D        # cols 0:64 <- mean of rows 0:64
    ones_ln[D:, D:] = 1.0 / D        # cols 64:128 <- mean of rows 64:128

    # l-major position offset: column s = l*BT + b carries value l*VOCAB
    posoff = np.repeat(np.arange(L) * VOCAB, BT)[None, :].repeat(NIDX, 0)

    c = lambda a: np.ascontiguousarray(np.asarray(a, np.float32))
    return {
        'stk': c(np.concatenate([E[0], E[1], E[2], E[3]], axis=1)),  # [42,168]
        'vw_a': c(np.concatenate([VW[0], VW[1]], axis=0)),           # [84,64]
        'vw_b': c(np.concatenate([VW[2], VW[3]], axis=0)),           # [84,64]
        'ones_ln': c(ones_ln),                                       # [128,128]
        'c1w': c(c1w),                                               # [64,128]
        'c2w': c(np.asarray(inp['e_c2w'])),                          # [128,64]
        'wf': c(Wf),                                                 # [64,11]
        'bo': c(np.asarray(inp['e_bo']).reshape(D, 1)),              # [64,1]
        'c1b': c(c1b.reshape(D2, 1)),                                # [128,1]
        'c2b': c(np.asarray(inp['e_c2b']).reshape(D, 1)),            # [64,1]
        'g1': c(g1.reshape(D, 1)),                                   # [64,1]
        'b1': c(b1.reshape(D, 1)),                                   # [64,1]
        'bf': c(bf.reshape(NOUT, 1)),                                # [11,1]
        'rep2': c(np.concatenate([np.eye(NIDX)] * 2, axis=1)),       # [42,84]
        'posoff': c(posoff),                                         # [42,448]
        'iota': c(np.arange(NIDX).reshape(NIDX, 1)),                 # [42,1]
    }


CONST_SPECS = [
    ('stk', [NIDX, H * NIDX], MMDT),
    ('vw_a', [2 * NIDX, D], MMDT),
    ('vw_b', [2 * NIDX, D], MMDT),
    ('ones_ln', [D2, D2], MMDT),
    ('c1w', [D, D2], MMDT),
    ('c2w', [D2, D], MMDT),
    ('wf', [D, NOUT], MMDT),
    ('bo', [D, 1], F32),
    ('c1b', [D2, 1], F32),
    ('c2b', [D, 1], F32),
    ('g1', [D, 1], F32),
    ('b1', [D, 1], F32),
    ('bf', [NOUT, 1], F32),
    ('rep2', [NIDX, 2 * NIDX], MMDT),
    ('posoff', [NIDX, NT], F32),
    ('iota', [NIDX, 1], F32),
]


def _ap(t: bass.AP, dims) -> bass.AP:
    """Rebuild an AP on t's storage: partition dim kept, free dims replaced
    by explicit [step, count] pairs (steps in elements, outer->inner)."""
    return bass.AP(tensor=t.tensor, offset=t.offset,
                   ap=[list(t.ap[0])] + [list(d) for d in dims])


def build_nc(ntiles: int = NTILES) -> bass.Bass:
    nc = bacc.Bacc()
    tokf = nc.declare_dram_parameter('tokf', [ntiles * NT], F32, isOutput=False)
    consts = {
        name: nc.declare_dram_parameter(name, shape, dtp, isOutput=False)
        for name, shape, dtp in CONST_SPECS
    }
    out_d = nc.declare_dram_parameter('out', [NOUT, ntiles * BT], F32, isOutput=True)

    with tile.TileContext(nc) as tc:
        with (
            tc.tile_pool(name='singles', bufs=1) as singles,
            tc.tile_pool(name='work', bufs=3) as work,
            tc.tile_pool(name='ps', bufs=1, space='PSUM') as ps,
        ):
            # --- load constants into SBUF once ---
            sb = {}
            for name, shape, dtp in CONST_SPECS:
                t = singles.tile(shape, dtp, tag=f'c_{name}')
                nc.sync.dma_start(out=t, in_=consts[name][:])
                sb[name] = t
            out_all = singles.tile([NOUT, ntiles * BT], F32)

            for it in range(ntiles):
                # ---- token one-hot + histogram ----
                tokrep = work.tile([NIDX, NT], F32, tag='tokrep')
                tok_slice = tokf[it * NT:(it + 1) * NT]
                nc.sync.dma_start(
                    out=tokrep,
                    in_=bass.AP(tensor=tok_slice.tensor, offset=tok_slice.offset,
                                ap=[[0, NIDX]] + list(tok_slice.ap)))
                idx = work.tile([NIDX, NT], F32, tag='idx')
                nc.vector.tensor_tensor(out=idx, in0=tokrep, in1=sb['posoff'],
                                        op=ALU.add)
                ohc = work.tile([NIDX, NT + BT], MMDT, tag='ohc')
                nc.vector.tensor_scalar(out=ohc[:, 0:NT], in0=idx,
                                        scalar1=sb['iota'], scalar2=None,
                                        op0=ALU.is_equal)
                # cnt[v, b] = sum_l oh[v, l*BT+b]  (reduce innermost strided l)
                with nc.allow_low_precision(reason='float32r is 4-byte fp32'):
                    nc.vector.tensor_reduce(
                        out=ohc[:, NT:NT + BT],
                        in_=_ap(ohc, [[1, BT], [BT, L]]),
                        axis=AX.X, op=ALU.add)

                # ---- per-head-pair Erow | W via one PE pass each ----
                be1 = ps.tile([2 * NIDX, NT + BT], F32, tag='be', bufs=3)
                be2 = ps.tile([2 * NIDX, NT + BT], F32, tag='be', bufs=3)
                nc.tensor.matmul(be1, sb['stk'][:, 0:2 * NIDX], ohc)
                nc.tensor.matmul(be2, sb['stk'][:, 2 * NIDX:4 * NIDX], ohc)

                # ---- R = cnt / W ; P = Erow * R (bcast over L) ----
                cnt2 = ps.tile([2 * NIDX, BT], F32, tag='st', bufs=1)
                nc.tensor.matmul(cnt2, sb['rep2'], ohc[:, NT:NT + BT])
                cnt2s = work.tile([2 * NIDX, BT], F32, tag='cnt2s')
                nc.scalar.copy(out=cnt2s, in_=cnt2)
                rw = work.tile([2 * NIDX, 2 * BT], F32, tag='rw')
                nc.vector.reciprocal(out=rw[:, 0:BT], in_=be1[:, NT:NT + BT])
                nc.vector.reciprocal(out=rw[:, BT:2 * BT], in_=be2[:, NT:NT + BT])
                R = work.tile([2 * NIDX, 2 * BT], F32, tag='R')
                nc.vector.tensor_tensor(
                    out=_ap(R, [[BT, 2], [1, BT]]),
                    in0=_ap(cnt2s, [[0, 2], [1, BT]]),
                    in1=_ap(rw, [[BT, 2], [1, BT]]), op=ALU.mult)
                P_a = work.tile([2 * NIDX, NT], MMDT, tag='P_a')
                P_b = work.tile([2 * NIDX, NT], MMDT, tag='P_b')
                nc.vector.tensor_tensor(
                    out=_ap(P_a, [[BT, L], [1, BT]]),
                    in0=_ap(be1, [[BT, L], [1, BT]]),
                    in1=_ap(R, [[0, L], [1, BT]]), op=ALU.mult)
                nc.vector.tensor_tensor(
                    out=_ap(P_b, [[BT, L], [1, BT]]),
                    in0=_ap(be2, [[BT, L], [1, BT]]),
                    in1=_ap(R[:, BT:2 * BT], [[0, L], [1, BT]]), op=ALU.mult)

                # ---- y = VW^T P (+bo) -> stacked1[:64]; x^2 -> [64:128] ----
                y_ps = ps.tile([D, NT], F32, tag='y', bufs=2)
                nc.tensor.matmul(y_ps, sb['vw_a'], P_a, start=True, stop=False)
                nc.tensor.matmul(y_ps, sb['vw_b'], P_b, start=False, stop=True)
                stk1 = work.tile([D2, NT], MMDT, tag='stk1')
                nc.vector.tensor_scalar(out=stk1[0:D, :], in0=y_ps,
                                        scalar1=sb['bo'], scalar2=None,
                                        op0=ALU.add)
                nc.scalar.square(out=stk1[D:D2, :], in_=stk1[0:D, :])

                # ---- LN1 stats via ones-matmul; normalize ----
                s1 = ps.tile([D2, NT], F32, tag='st', bufs=1)
                nc.tensor.matmul(s1, sb['ones_ln'], stk1)
                msq1 = work.tile([D, NT], F32, tag='msq1')
                nc.scalar.square(out=msq1, in_=s1[0:D, :])
                veps1 = work.tile([D, NT], F32, tag='veps1')
                nc.vector.scalar_tensor_tensor(out=veps1, in0=s1[D:D2, :],
                                               scalar=float(EPS), in1=msq1,
                                               op0=ALU.add, op1=ALU.subtract)
                rvep1 = work.tile([D, NT], F32, tag='rvep1')
                nc.vector.reciprocal(out=rvep1, in_=veps1)
                rstd1 = work.tile([D, NT], F32, tag='rstd1')
                nc.scalar.sqrt(out=rstd1, in_=rvep1)
                xs1 = work.tile([D, NT], F32, tag='xs1')
                nc.vector.tensor_tensor(out=xs1, in0=stk1[0:D, :], in1=s1[0:D, :],
                                        op=ALU.subtract)
                xc = work.tile([D, NT], MMDT, tag='xc')
                nc.vector.tensor_tensor(out=xc, in0=xs1, in1=rstd1, op=ALU.mult)

                # ---- FFN (LN1 affine folded into c1w/c1b) ----
                c1_ps = ps.tile([D2, NT], F32, tag='c1', bufs=1)
                nc.tensor.matmul(c1_ps, sb['c1w'], xc)
                x1 = work.tile([D2, NT], MMDT, tag='x1')
                nc.vector.tensor_scalar(out=x1, in0=c1_ps, scalar1=sb['c1b'],
                                        scalar2=0.0, op0=ALU.add, op1=ALU.max)
                c2_ps = ps.tile([D, NT], F32, tag='c2', bufs=1)
                nc.tensor.matmul(c2_ps, sb['c2w'], x1)
                x2 = work.tile([D, NT], F32, tag='x2')
                nc.vector.tensor_scalar(out=x2, in0=c2_ps, scalar1=sb['c2b'],
                                        scalar2=0.0, op0=ALU.add, op1=ALU.max)

                # ---- z = g1*xc + b1 + x2 ; LN2 ----
                stk2 = work.tile([D2, NT], MMDT, tag='stk2')
                zaff = work.tile([D, NT], F32, tag='zaff')
                nc.vector.tensor_scalar(out=zaff, in0=xc, scalar1=sb['g1'],
                                        scalar2=sb['b1'], op0=ALU.mult,
                                        op1=ALU.add)
                nc.vector.tensor_tensor(out=stk2[0:D, :], in0=zaff, in1=x2,
                                        op=ALU.add)
                nc.scalar.square(out=stk2[D:D2, :], in_=stk2[0:D, :])
                s2 = ps.tile([D2, NT], F32, tag='st', bufs=1)
                nc.tensor.matmul(s2, sb['ones_ln'], stk2)
                msq2 = work.tile([D, NT], F32, tag='msq2')
                nc.scalar.square(out=msq2, in_=s2[0:D, :])
                veps2 = work.tile([D, NT], F32, tag='veps2')
                nc.vector.scalar_tensor_tensor(out=veps2, in0=s2[D:D2, :],
                                               scalar=float(EPS), in1=msq2,
                                               op0=ALU.add, op1=ALU.subtract)
                rvep2 = work.tile([D, NT], F32, tag='rvep2')
                nc.vector.reciprocal(out=rvep2, in_=veps2)
                rstd2 = work.tile([D, NT], F32, tag='rstd2')
                nc.scalar.sqrt(out=rstd2, in_=rvep2)
                zs = work.tile([D, NT], F32, tag='zs')
                nc.vector.tensor_tensor(out=zs, in0=stk2[0:D, :], in1=s2[0:D, :],
                                        op=ALU.subtract)
                u7 = work.tile([D, NT], F32, tag='u7')
                nc.vector.tensor_tensor(out=u7, in0=zs, in1=rstd2, op=ALU.mult)

                # ---- per-item reduce over L, fused decoder+fc matmul ----
                ured = work.tile([D, BT], MMDT, tag='ured')
                with nc.allow_low_precision(reason='float32r is 4-byte fp32'):
                    nc.vector.tensor_reduce(
                        out=ured, in_=_ap(u7, [[1, BT], [BT, L]]),
                        axis=AX.X, op=ALU.add)
                o_ps = ps.tile([NOUT, BT], F32, tag='st', bufs=1)
                nc.tensor.matmul(o_ps, sb['wf'], ured)
                nc.vector.tensor_scalar(out=out_all[:, it * BT:(it + 1) * BT],
                                        in0=o_ps, scalar1=sb['bf'], scalar2=None,
                                        op0=ALU.add)

            nc.sync.dma_start(out=out_d[:], in_=out_all)
    nc.compile()
    return nc


# ---------------- host entry point ----------------
_CACHE: dict = {}


def shard_tokens(tokens: np.ndarray) -> np.ndarray:
    """[B', L] int -> [NCORES, ntiles*NT] f32, l-major within each tile."""
    t = tokens.astype(np.float32).reshape(NCORES, -1, BT, L)
    t = t.transpose(0, 1, 3, 2)                  # [NC, ntiles, L, BT]
    return np.ascontiguousarray(t.reshape(NCORES, -1))


def kernel(**inputs) -> np.ndarray:
    tokens = np.asarray(inputs['tokens'])
    assert tokens.shape == (B, L)
    tb = build_tables(inputs)
    if 'nc' not in _CACHE:
        _CACHE['nc'] = build_nc(NTILES)
    nc = _CACHE['nc']
    tokf = shard_tokens(tokens)
    in_maps = []
    for c in range(NCORES):
        m = {'tokf': tokf[c]}
        m.update({name: tb[name] for name, _, _ in CONST_SPECS})
        in_maps.append(m)
    res = run_bass_kernel_spmd(nc, in_maps, list(range(NCORES)))
    outs = [res.results[c]['out'].T for c in range(NCORES)]   # [BC, NOUT] each
    return np.ascontiguousarray(np.concatenate(outs, axis=0).astype(np.float32))
